# revision 6
# baseline (speedup 1.0000x reference)
"""Trainium2 Bass kernel for nn_Attn: out = softmax(v . (W @ q_s + b)) over s.

Key algebraic identity:
    energies[s] = v . (W @ q[s] + b) = q[s] . (W^T v) + (v . b)
The (v . b) term is constant across s and softmax is shift-invariant, so it
drops out. The kernel therefore computes u = W^T v (tiny), then a matvec
energies = question @ u, then a global softmax. This is memory-bound:
question (128 MiB fp32) must stream from HBM once; everything else is noise.

Distribution over 8 NeuronCores — sequence (s) sharding:
  - core r owns tokens [r*4096, (r+1)*4096); host sends it the transposed
    slab QT[:, r*4096:(r+1)*4096] = [1024, 4096] as 8 contiguous h-chunks
    of [128, 4096] (2 MB DMAs, first thing on the sync queue)
  - u = W^T v, obtained per U_MODE:
      "ag": core r computes its slice u_r from the W column-slab, then a
            tiny AllGather (512 B/core) overlapped with the q stream
      "rep": every core streams the full W (4 MB, replicated) and computes
            all of u locally — no early collective on the critical path
  - energy matmuls contract h chunk-by-chunk, accumulating in PSUM
    (start/stop flags); fp32 matmuls are 2-pass so four M=1 matmuls are
    packed into the four 32-column groups of the PE array (tile_position
    with psum base partitions 0/32/64/96); 8 token-groups of 512 across
    2 PSUM banks. Each core ends with COMPLETE energies for its 4096
    tokens — no ReduceScatter needed.
  - one AllGather (16 KB in / 128 KB out) distributes the full energy
    vector; each core computes the identical global softmax with
    per-partition max/sum stats; host takes core 0's output.
"""

import numpy as np

S = 32768
H = 1024
NCORES = 8
SL = S // NCORES  # 4096 tokens per core
HC = H // 128  # 8 h-chunks of 128
NGL = SL // 512  # 8 token groups of 512 per core

U_MODE = "ag"  # "ag" (u-slice + AllGather) or "rep" (replicated W)
DEBUG = True  # extra tiny outputs (u, local energies) for bisection

_cached = {}


def _build():
    """Build + compile the SPMD Bass module (same NEFF on all 8 cores)."""
    from contextlib import ExitStack

    import concourse.bass as bass
    import concourse.mybir as mybir
    import concourse.tile as tile
    from concourse import bacc
    from concourse.masks import make_identity

    f32 = mybir.dt.float32
    AX = mybir.AxisListType
    OP = mybir.AluOpType
    ds = bass.ds

    nc = bacc.Bacc(
        "TRN2", target_bir_lowering=False, debug=False, num_devices=NCORES
    )

    qts = nc.dram_tensor("qts", [H, SL], f32, kind="ExternalInput")
    if U_MODE == "ag":
        wc = nc.dram_tensor("wc", [H, 128], f32, kind="ExternalInput")
        vt = nc.dram_tensor("vt", [128, HC], f32, kind="ExternalInput")
    else:
        wf = nc.dram_tensor("wf", [H, H], f32, kind="ExternalInput")
        vt = nc.dram_tensor("vt", [128, HC], f32, kind="ExternalInput")
    out = nc.dram_tensor("out", [S], f32, kind="ExternalOutput")
    if DEBUG:
        dbg_u = nc.dram_tensor("dbg_u", [NCORES, 128] if U_MODE == "ag" else [1, H],
                               f32, kind="ExternalOutput")
        dbg_e = nc.dram_tensor("dbg_e", [1, SL], f32, kind="ExternalOutput")

    rg = [list(range(NCORES))]

    with tile.TileContext(nc) as tc, ExitStack() as ctx:
        const = ctx.enter_context(tc.tile_pool(name="const", bufs=1))
        qpool = ctx.enter_context(tc.tile_pool(name="qpool", bufs=HC))
        work = ctx.enter_context(tc.tile_pool(name="work", bufs=1))
        psum_e = ctx.enter_context(tc.tile_pool(name="psum_e", bufs=2, space="PSUM"))
        psum_s = ctx.enter_context(tc.tile_pool(name="psum_s", bufs=2, space="PSUM"))
        dram = ctx.enter_context(tc.tile_pool(name="dram", bufs=1, space="DRAM"))

        # ---- question stream: 8 chunk DMAs, FIRST on the sync queue ----
        q_sb = []
        for c in range(HC):
            q = qpool.tile([128, SL], f32, tag="q", name=f"q{c}")
            nc.sync.dma_start(q[:], qts[c * 128 : (c + 1) * 128, :])
            q_sb.append(q)

        # ---- small inputs on the scalar (Activation) HWDGE queue ----
        v_sb = const.tile([128, HC], f32)
        nc.scalar.dma_start(v_sb[:], vt[:])

        ident = const.tile([128, 128], f32)
        make_identity(nc, ident[:])
        # pre-warm the Exp activation table while the stream runs (the
        # table load is ~1.5 us; doing it now keeps it off the tail)
        warm = work.tile([1, 1], f32)
        nc.scalar.activation(
            warm[:], v_sb[0:1, 0:1], mybir.ActivationFunctionType.Exp
        )

        u_sb = const.tile([128, HC], f32)  # u as [j_in_chunk, chunk]

        if U_MODE == "ag":
            # wc viewed o=(p c): per-partition contiguous 4 KB (128 descs).
            # v must use the matching (p c) order: v_sb[p, c] = v[p*8+c].
            wc_sb = const.tile([128, HC * 128], f32)
            nc.scalar.dma_start(
                wc_sb[:], wc[:].rearrange("(p c) m -> p (c m)", p=128)
            )
            # local u-slice: u_r[j] = sum_o W[o, r*128+j] v[o]
            pu = psum_s.tile([128, 1], f32, tag="stat")
            for c in range(HC):
                nc.tensor.matmul(
                    pu[:], lhsT=wc_sb[:, c * 128 : (c + 1) * 128],
                    rhs=v_sb[:, c : c + 1],
                    start=(c == 0), stop=(c == HC - 1),
                )
            u_loc = const.tile([128, 1], f32)
            nc.vector.tensor_copy(u_loc[:], pu[:])
            # fat-descriptor export: transpose on TensorE, 512 B DMA
            ptu = psum_s.tile([1, 128], f32, tag="statb")
            nc.tensor.transpose(ptu[:], u_loc[:], ident[:])
            u_row = const.tile([1, 128], f32)
            nc.vector.tensor_copy(u_row[:], ptu[:])
            u_r_dram = dram.tile([1, 128], f32)
            nc.scalar.dma_start(u_r_dram[:], u_row[:])
            u_all_dram = dram.tile([NCORES, 128], f32)
            nc.gpsimd.collective_compute(
                "AllGather", OP.bypass, replica_groups=rg,
                ins=[u_r_dram.opt()], outs=[u_all_dram.opt()],
            )
            u_all_sb = const.tile([NCORES, 128], f32)
            nc.scalar.dma_start(u_all_sb[:], u_all_dram[:])
            ptv = psum_s.tile([128, HC], f32, tag="stat")
            nc.tensor.transpose(ptv[:], u_all_sb[:], ident[0:NCORES, 0:NCORES])
            nc.vector.tensor_copy(u_sb[:], ptv[:])
        else:
            # replicated W: u[j] = sum_o v[o] W[o, j] via M=1 matmuls with
            # W as the streamed rhs ((c p)-major view, 4 KB descriptors)
            wf_sb = const.tile([128, HC, H], f32)
            nc.scalar.dma_start(
                wf_sb[:], wf[:].rearrange("(c p) j -> p c j", p=128)
            )
            pua = psum_s.tile([1, 512], f32, tag="stat")
            pub = psum_s.tile([1, 512], f32, tag="statb")
            for c in range(HC):
                nc.tensor.matmul(
                    pua[:], lhsT=v_sb[:, c : c + 1],
                    rhs=wf_sb[:, c, 0:512],
                    start=(c == 0), stop=(c == HC - 1),
                )
                nc.tensor.matmul(
                    pub[:], lhsT=v_sb[:, c : c + 1],
                    rhs=wf_sb[:, c, 512:1024],
                    start=(c == 0), stop=(c == HC - 1),
                )
            u_row2 = const.tile([1, H], f32)
            nc.vector.tensor_copy(u_row2[0:1, 0:512], pua[:])
            nc.vector.tensor_copy(u_row2[0:1, 512:1024], pub[:])
            # 8 tiny transposes [1,128] -> [128,1] to build u_sb columns
            for c in range(HC):
                ptc = psum_s.tile([128, 1], f32, tag="stat")
                nc.tensor.transpose(
                    ptc[:], u_row2[0:1, c * 128 : (c + 1) * 128], ident[0:1, 0:1]
                )
                nc.vector.tensor_copy(u_sb[:, c : c + 1], ptc[:])

        # ---- complete energies for this core's 4096 tokens ----
        # 8 groups of 512 across 2 PSUM banks; within a bank the four M=1
        # fp32 matmuls overlap in the PE column groups (tile_position from
        # the psum slice's base partition 0/32/64/96). h contracted
        # chunk-by-chunk via PSUM accumulation (start at c=0, stop at c=7).
        peA = psum_e.tile([128, 512], f32, tag="peA")
        peB = psum_e.tile([128, 512], f32, tag="peB")
        for c in range(HC):
            for g in range(NGL):
                tgt = peA if g < 4 else peB
                j = g % 4
                nc.tensor.matmul(
                    tgt[32 * j : 32 * j + 1, :],
                    lhsT=u_sb[:, c : c + 1],
                    rhs=q_sb[c][:, ds(g * 512, 512)],
                    start=(c == 0), stop=(c == HC - 1),
                    tile_position=(0, 32 * j),
                )

        # engines can't read strided partitions; copy the full banks and
        # let the export DMAs stride instead (sync queue is idle by now)
        e_loc_dram = dram.tile([1, SL], f32)
        e_view = e_loc_dram[:].rearrange("one (g s) -> (one g) s", s=512)
        esbA = work.tile([128, 512], f32)
        nc.vector.tensor_copy(esbA[:], peA[:])
        esbB = work.tile([128, 512], f32)
        nc.scalar.copy(esbB[:], peB[:])
        rowsA = esbA[:].rearrange("(a b) s -> a b s", b=32)
        rowsB = esbB[:].rearrange("(a b) s -> a b s", b=32)
        nc.sync.dma_start(e_view[0:4, :], rowsA[:, 0, :])
        nc.sync.dma_start(e_view[4:8, :], rowsB[:, 0, :])

        # ---- AllGather the full energy vector (16 KB in, 128 KB out) ----
        e_sum_dram = dram.tile([NCORES, SL], f32)
        nc.gpsimd.collective_compute(
            "AllGather", OP.bypass, replica_groups=rg,
            ins=[e_loc_dram.opt()], outs=[e_sum_dram.opt()],
        )

        # ---- global softmax over all 32768 energies ----
        # Layout [128, 256]: per-partition stats first (no broadcasts), then
        # one tiny transpose to combine across partitions and one to come back.
        F = S // 128  # 256
        e_all = work.tile([128, F], f32)
        nc.sync.dma_start(
            e_all[:], e_sum_dram[:].rearrange("r (q f) -> (r q) f", f=F)
        )
        negrow = work.tile([128, 1], f32)
        nc.vector.tensor_reduce(negrow[:], e_all[:], axis=AX.X, op=OP.max, negate=True)
        ex1 = work.tile([128, F], f32)
        rowsum = work.tile([128, 1], f32)
        nc.scalar.activation(
            ex1[:], e_all[:], mybir.ActivationFunctionType.Exp,
            bias=negrow[:], scale=1.0, accum_out=rowsum[:],
        )
        ptr_a = psum_s.tile([1, 128], f32, tag="stat")
        nc.tensor.transpose(ptr_a[:], negrow[:], ident[:])
        ptr_b = psum_s.tile([1, 128], f32, tag="statb")
        nc.tensor.transpose(ptr_b[:], rowsum[:], ident[:])
        tp0 = work.tile([1, 128], f32)
        nc.vector.tensor_copy(tp0[:], ptr_a[:])
        tp1 = work.tile([1, 128], f32)
        nc.scalar.copy(tp1[:], ptr_b[:])
        # global stats on one partition: m = max_j rowmax_j, s = sum_j
        # rowsum_j * exp(rowmax_j - m); tp0 holds -rowmax_j, tp1 rowsum_j
        negm = work.tile([1, 1], f32)
        nc.vector.tensor_reduce(negm[:], tp0[:], axis=AX.X, op=OP.min)
        texp = work.tile([1, 128], f32)
        nc.scalar.activation(
            texp[:], tp0[:], mybir.ActivationFunctionType.Exp,
            bias=negm[:], scale=-1.0,
        )
        prod = work.tile([1, 128], f32)
        nc.vector.tensor_mul(prod[:], texp[:], tp1[:])
        stot = work.tile([1, 1], f32)
        nc.vector.tensor_reduce(stot[:], prod[:], axis=AX.X, op=OP.add)
        rtot = work.tile([1, 1], f32)
        nc.vector.reciprocal(rtot[:], stot[:])
        # K=1 matmul does transpose + scale in one: scl[j] = texp[j] / s
        pscl = psum_s.tile([128, 1], f32, tag="statb")
        nc.tensor.matmul(pscl[:], lhsT=texp[:], rhs=rtot[:], start=True, stop=True)
        scl = work.tile([128, 1], f32)
        nc.vector.tensor_copy(scl[:], pscl[:])
        outt = work.tile([128, F], f32)
        nc.vector.tensor_scalar_mul(outt[:], ex1[:], scl[:])
        nc.sync.dma_start(out[:].rearrange("(p f) -> p f", f=F), outt[:])

        if DEBUG:
            # tiny end-of-kernel dumps for bisection (DRAM -> DRAM)
            if U_MODE == "ag":
                nc.sync.dma_start(dbg_u[:], u_all_dram[:])
            else:
                nc.sync.dma_start(dbg_u[:], u_row2[:])
            nc.sync.dma_start(dbg_e[:], e_loc_dram[:])

    nc.compile()
    return nc


def _get_nc():
    if "nc" not in _cached:
        _cached["nc"] = _build()
    return _cached["nc"]


def make_in_maps(question, W, v):
    q = np.ascontiguousarray(np.asarray(question, dtype=np.float32))
    Wn = np.ascontiguousarray(np.asarray(W, dtype=np.float32))
    vn = np.ascontiguousarray(np.asarray(v, dtype=np.float32))
    in_maps = []
    for r in range(NCORES):
        m = {"qts": np.ascontiguousarray(q[r * SL : (r + 1) * SL, :].T)}
        if U_MODE == "ag":
            # (p c)-major contraction order: v_sb[p, c] = v[p*8 + c]
            m["vt"] = np.ascontiguousarray(vn.reshape(128, HC))
            m["wc"] = np.ascontiguousarray(Wn[:, r * 128 : (r + 1) * 128])
        else:
            # (c p)-major contraction order: v_sb[p, c] = v[c*128 + p]
            m["vt"] = np.ascontiguousarray(vn.reshape(HC, 128).T)
            m["wf"] = Wn
        in_maps.append(m)
    return in_maps


def run(question, W, v, **spmd_kwargs):
    """Run the SPMD kernel; returns (out [S] fp32, BassKernelResults)."""
    from concourse.bass_utils import run_bass_kernel_spmd

    nc = _get_nc()
    in_maps = make_in_maps(question, W, v)
    res = run_bass_kernel_spmd(nc, in_maps, core_ids=list(range(NCORES)), **spmd_kwargs)
    return np.asarray(res.results[0]["out"], dtype=np.float32), res


def kernel(question, W, b, v):
    out, _ = run(question, W, v)
    return out.reshape(1, 1, S)


# revision 8
# speedup vs baseline: 1.1921x; 1.1921x over previous
"""Trainium2 Bass kernel for nn_Attn: out = softmax(v . (W @ q_s + b)) over s.

Key algebraic identity:
    energies[s] = v . (W @ q[s] + b) = q[s] . (W^T v) + (v . b)
The (v . b) term is constant across s and softmax is shift-invariant, so it
drops out. The kernel computes u = W^T v (tiny), then a matvec
energies = question @ u, then a global softmax. This is memory-bound:
question (128 MiB fp32) must stream from HBM once; everything else is noise.

Distribution over 8 NeuronCores — sequence (s) sharding:
  - core r owns tokens [r*4096, (r+1)*4096); host sends the transposed slab
    QT[:, r*4096:(r+1)*4096] reshaped [128, 8, 4096] so chunk c holds
    h = p*8 + c on partition p (contiguous 16 KB descriptors)
  - W is REPLICATED, with v folded in as column 0 of each chunk row
    (W_aug [128, 8, 1025], 32 KB descriptors, first on the single queue):
    ncfw collectives have a ~40-60 us per-kernel setup floor, so an early
    u-AllGather can never beat just streaming 4 extra MB of W
  - u = W^T v via 16 M=1 matmuls (rhs = W rows, 2 PSUM banks column-packed),
    then 8 tiny transposes build u_sb[p, c] = u[p*8+c]
  - energy matmuls contract h chunk-by-chunk, accumulating in PSUM
    (start/stop); four M=1 fp32 matmuls pack into the four 32-column PE
    groups (tile_position, psum base partitions 0/32/64/96); 8 token-groups
    of 512 across 2 PSUM banks. Each core ends with COMPLETE energies for
    its 4096 tokens — no ReduceScatter.
  - one AllGather (16 KB in / 128 KB out) distributes the energy vector;
    every core computes the identical global softmax; host takes core 0.
"""

import numpy as np

S = 32768
H = 1024
NCORES = 8
SL = S // NCORES  # 4096 tokens per core
HC = H // 128  # 8 h-chunks of 128
NGL = SL // 512  # 8 token groups of 512 per core
WR = H + 1  # W_aug row length: [v | W row]

DEBUG = True  # extra tiny outputs (u, local energies) for bisection

_cached = {}


def _build():
    """Build + compile the SPMD Bass module (same NEFF on all 8 cores)."""
    from contextlib import ExitStack

    import concourse.bass as bass
    import concourse.mybir as mybir
    import concourse.tile as tile
    from concourse import bacc
    from concourse.masks import make_identity

    f32 = mybir.dt.float32
    AX = mybir.AxisListType
    OP = mybir.AluOpType
    ds = bass.ds

    nc = bacc.Bacc(
        "TRN2", target_bir_lowering=False, debug=False, num_devices=NCORES
    )

    qts = nc.dram_tensor("qts", [128, HC, SL], f32, kind="ExternalInput")
    waug = nc.dram_tensor("waug", [128, HC, WR], f32, kind="ExternalInput")
    out = nc.dram_tensor("out", [S], f32, kind="ExternalOutput")
    if DEBUG:
        dbg_u = nc.dram_tensor("dbg_u", [1, H], f32, kind="ExternalOutput")
        dbg_e = nc.dram_tensor("dbg_e", [1, SL], f32, kind="ExternalOutput")

    rg = [list(range(NCORES))]

    with tile.TileContext(nc) as tc, ExitStack() as ctx:
        const = ctx.enter_context(tc.tile_pool(name="const", bufs=1))
        qpool = ctx.enter_context(tc.tile_pool(name="qpool", bufs=HC))
        work = ctx.enter_context(tc.tile_pool(name="work", bufs=1))
        psum_e = ctx.enter_context(tc.tile_pool(name="psum_e", bufs=2, space="PSUM"))
        psum_s = ctx.enter_context(tc.tile_pool(name="psum_s", bufs=2, space="PSUM"))
        dram = ctx.enter_context(tc.tile_pool(name="dram", bufs=1, space="DRAM"))

        # ---- single HWDGE queue, fat descriptors only: W first, then q ----
        w_sb = const.tile([128, HC, WR], f32)
        nc.sync.dma_start(w_sb[:], waug[:])
        q_sb = []
        for c in range(HC):
            q = qpool.tile([128, SL], f32, tag="q", name=f"q{c}")
            nc.sync.dma_start(q[:], qts[:, c, :])
            q_sb.append(q)

        ident = const.tile([128, 128], f32)
        make_identity(nc, ident[:])
        # pre-warm the Exp table while the stream runs (~1.5 us off the tail)
        warm = work.tile([1, 1], f32)
        nc.scalar.activation(
            warm[:], w_sb[0:1, 0, 0:1], mybir.ActivationFunctionType.Exp
        )

        # ---- u = W^T v: chunk c contracts o = p*8+c over partitions p ----
        # lhsT = v column (w_sb[:, c, 0]), rhs = W rows; j in 2 psum banks
        # packed in different PE column groups so the 2-pass fp32 matmuls
        # overlap.
        pua = psum_s.tile([1, 512], f32, tag="stat")
        pub = psum_s.tile([1, 512], f32, tag="statb")
        for c in range(HC):
            nc.tensor.matmul(
                pua[:], lhsT=w_sb[:, c, 0:1], rhs=w_sb[:, c, 1 : 1 + 512],
                start=(c == 0), stop=(c == HC - 1),
            )
            nc.tensor.matmul(
                pub[:], lhsT=w_sb[:, c, 0:1], rhs=w_sb[:, c, 513 : 513 + 512],
                start=(c == 0), stop=(c == HC - 1),
            )
        u_row = const.tile([1, H], f32)
        nc.vector.tensor_copy(u_row[0:1, 0:512], pua[:])
        nc.scalar.copy(u_row[0:1, 512:1024], pub[:])
        # u_sb[p, c] = u[p*8+c]: transpose strided [1,128] views back to
        # partition-major columns
        u_sb = const.tile([128, HC], f32)
        u_pc = u_row[:].rearrange("one (p c) -> one p c", c=HC)
        for c in range(HC):
            ptc = psum_s.tile([128, 1], f32, tag="stat" if c % 2 == 0 else "statb")
            nc.tensor.transpose(ptc[:], u_pc[:, :, c], ident[0:1, 0:1])
            if c % 2 == 0:
                nc.vector.tensor_copy(u_sb[:, c : c + 1], ptc[:])
            else:
                nc.scalar.copy(u_sb[:, c : c + 1], ptc[:])

        # ---- complete energies for this core's 4096 tokens ----
        peA = psum_e.tile([128, 512], f32, tag="peA")
        peB = psum_e.tile([128, 512], f32, tag="peB")
        for c in range(HC):
            for g in range(NGL):
                tgt = peA if g < 4 else peB
                j = g % 4
                nc.tensor.matmul(
                    tgt[32 * j : 32 * j + 1, :],
                    lhsT=u_sb[:, c : c + 1],
                    rhs=q_sb[c][:, ds(g * 512, 512)],
                    start=(c == 0), stop=(c == HC - 1),
                    tile_position=(0, 32 * j),
                )

        # engines can't read strided partitions; copy the full banks and
        # let the export DMAs stride instead (queue is idle by now)
        e_loc_dram = dram.tile([1, SL], f32)
        e_view = e_loc_dram[:].rearrange("one (g s) -> (one g) s", s=512)
        esbA = work.tile([128, 512], f32)
        nc.vector.tensor_copy(esbA[:], peA[:])
        esbB = work.tile([128, 512], f32)
        nc.scalar.copy(esbB[:], peB[:])
        rowsA = esbA[:].rearrange("(a b) s -> a b s", b=32)
        rowsB = esbB[:].rearrange("(a b) s -> a b s", b=32)
        nc.sync.dma_start(e_view[0:4, :], rowsA[:, 0, :])
        nc.sync.dma_start(e_view[4:8, :], rowsB[:, 0, :])

        # ---- AllGather the full energy vector (16 KB in, 128 KB out) ----
        e_sum_dram = dram.tile([NCORES, SL], f32)
        nc.gpsimd.collective_compute(
            "AllGather", OP.bypass, replica_groups=rg,
            ins=[e_loc_dram.opt()], outs=[e_sum_dram.opt()],
        )

        # ---- global softmax over all 32768 energies ----
        F = S // 128  # 256
        e_all = work.tile([128, F], f32)
        nc.sync.dma_start(
            e_all[:], e_sum_dram[:].rearrange("r (q f) -> (r q) f", f=F)
        )
        negrow = work.tile([128, 1], f32)
        nc.vector.tensor_reduce(negrow[:], e_all[:], axis=AX.X, op=OP.max, negate=True)
        ex1 = work.tile([128, F], f32)
        rowsum = work.tile([128, 1], f32)
        nc.scalar.activation(
            ex1[:], e_all[:], mybir.ActivationFunctionType.Exp,
            bias=negrow[:], scale=1.0, accum_out=rowsum[:],
        )
        ptr_a = psum_s.tile([1, 128], f32, tag="stat")
        nc.tensor.transpose(ptr_a[:], negrow[:], ident[:])
        ptr_b = psum_s.tile([1, 128], f32, tag="statb")
        nc.tensor.transpose(ptr_b[:], rowsum[:], ident[:])
        tp0 = work.tile([1, 128], f32)
        nc.vector.tensor_copy(tp0[:], ptr_a[:])
        tp1 = work.tile([1, 128], f32)
        nc.scalar.copy(tp1[:], ptr_b[:])
        # global stats on one partition: m = max_j rowmax_j, s = sum_j
        # rowsum_j * exp(rowmax_j - m); tp0 holds -rowmax_j, tp1 rowsum_j
        negm = work.tile([1, 1], f32)
        nc.vector.tensor_reduce(negm[:], tp0[:], axis=AX.X, op=OP.min)
        texp = work.tile([1, 128], f32)
        nc.scalar.activation(
            texp[:], tp0[:], mybir.ActivationFunctionType.Exp,
            bias=negm[:], scale=-1.0,
        )
        prod = work.tile([1, 128], f32)
        nc.vector.tensor_mul(prod[:], texp[:], tp1[:])
        stot = work.tile([1, 1], f32)
        nc.vector.tensor_reduce(stot[:], prod[:], axis=AX.X, op=OP.add)
        rtot = work.tile([1, 1], f32)
        nc.vector.reciprocal(rtot[:], stot[:])
        # K=1 matmul does transpose + scale in one: scl[j] = texp[j] / s
        pscl = psum_s.tile([128, 1], f32, tag="statb")
        nc.tensor.matmul(pscl[:], lhsT=texp[:], rhs=rtot[:], start=True, stop=True)
        scl = work.tile([128, 1], f32)
        nc.vector.tensor_copy(scl[:], pscl[:])
        outt = work.tile([128, F], f32)
        nc.vector.tensor_scalar_mul(outt[:], ex1[:], scl[:])
        nc.sync.dma_start(out[:].rearrange("(p f) -> p f", f=F), outt[:])

        if DEBUG:
            nc.sync.dma_start(dbg_u[:], u_row[:])
            nc.sync.dma_start(dbg_e[:], e_loc_dram[:])

    nc.compile()
    return nc


def _get_nc():
    if "nc" not in _cached:
        _cached["nc"] = _build()
    return _cached["nc"]


def make_in_maps(question, W, v):
    q = np.ascontiguousarray(np.asarray(question, dtype=np.float32))
    Wn = np.ascontiguousarray(np.asarray(W, dtype=np.float32))
    vn = np.ascontiguousarray(np.asarray(v, dtype=np.float32))
    # W_aug[p, c, 0] = v[p*8+c]; W_aug[p, c, 1+j] = W[p*8+c, j]
    waug = np.empty((128, HC, WR), dtype=np.float32)
    waug[:, :, 0] = vn.reshape(128, HC)
    waug[:, :, 1:] = Wn.reshape(128, HC, H)
    in_maps = []
    for r in range(NCORES):
        # qts[p, c, s] = q[r*SL+s, p*8+c]
        qt = np.ascontiguousarray(q[r * SL : (r + 1) * SL, :].T)  # [H, SL]
        in_maps.append({"qts": qt.reshape(128, HC, SL), "waug": waug})
    return in_maps


def run(question, W, v, **spmd_kwargs):
    """Run the SPMD kernel; returns (out [S] fp32, BassKernelResults)."""
    from concourse.bass_utils import run_bass_kernel_spmd

    nc = _get_nc()
    in_maps = make_in_maps(question, W, v)
    res = run_bass_kernel_spmd(nc, in_maps, core_ids=list(range(NCORES)), **spmd_kwargs)
    return np.asarray(res.results[0]["out"], dtype=np.float32), res


def kernel(question, W, b, v):
    out, _ = run(question, W, v)
    return out.reshape(1, 1, S)
